# revision 6
# baseline (speedup 1.0000x reference)
"""DAGNN (gnn_message_passing) Trainium2 Bass kernel, 8-core SPMD.

Strategy:
  - Nodes padded 100000 -> 100352 (784*128); 8 dst-shards of 12544 rows.
  - Symmetric norm factored into per-node scales:
        feat_{k+1} = Di A Do feat_k,  Di=diag(dinv_in), Do=diag(dinv_out)
    We keep g_k = Do feat_k as the gathered table, so the edge aggregation
    is an unweighted segment-sum; per-node scales are applied on the shard
    before the per-hop AllGather.
  - Edges of each core sorted by dst, packed into 128-edge tiles confined
    to 64-dst windows.  Per tile: indirect-DMA row gather from the HBM
    table, a batched DVE is_equal builds the 0/1 one-hot [128e x 64d], and
    a PE matmul (lhsT=onehot, rhs=msg) scatter-accumulates into PSUM.
  - MLP (512->64 relu ->64 relu) on PE; adaptive gate on DVE/ACT.
"""

import os
import sys

for p in ("/opt/trn_rl_repo", "/root/.axon_site/_ro/trn_rl_repo"):
    if os.path.isdir(p) and p not in sys.path:
        sys.path.insert(0, p)

import numpy as np

import concourse.bass as bass
import concourse.tile as tile
from concourse import mybir
from concourse.bass import AP, IndirectOffsetOnAxis
from concourse.bass_utils import run_bass_kernel_spmd
from concourse.masks import make_identity
from concourse.vector_clock import ScopedClock

# ----------------------------------------------------------------------------
# problem constants (hardcoded per contract)
N_NODES = 100000
N_EDGES = 3200000
IN_F, HID, OUT_F = 512, 64, 64
K_HOPS = 10
N_CORES = 8

NP_PAD = 100352            # 784 * 128
PER = NP_PAD // N_CORES    # 12544 = 98 * 128
BLOCKS = PER // 128        # 98
WIN = 64                   # dsts per one-hot window
N_WIN = PER // WIN         # 196 windows per core
F = OUT_F                  # 64

_f32 = mybir.dt.float32
_i32 = mybir.dt.int32


# ----------------------------------------------------------------------------
# This walrus build supports at most ONE sync wait per instruction.  After
# Tile has scheduled and assigned semaphores, split any instruction carrying
# N>1 waits into (N-1) same-engine NOPs (engine streams are FIFO, so waits on
# preceding nops gate the instruction identically) + the instruction with 1.
_WSPLIT_CTR = [0]


def split_excess_waits(nc):
    n_split = 0
    for bb in nc.main_func.blocks:
        il = bb.instructions  # live list
        i = 0
        while i < len(il):
            inst = il[i]
            si = inst.sync_info
            if si is None:
                i += 1
                continue
            waits = list(si.on_wait)
            if len(waits) <= 1:
                i += 1
                continue
            for w in waits[:-1]:
                _WSPLIT_CTR[0] += 1
                nop = mybir.InstNoOp(
                    name=f"WSPLIT-{_WSPLIT_CTR[0]}", ins=[], outs=[]
                )
                nop.engine = inst.engine
                nop.sync_info = mybir.SyncInfo(on_wait=[w], on_update=[])
                nc.register_instruction(nop, overwrite=True)
                il.insert(i, nop)
                i += 1
            inst.sync_info = mybir.SyncInfo(
                on_wait=[waits[-1]], on_update=list(si.on_update)
            )
            n_split += 1
            i += 1
    return n_split


# ----------------------------------------------------------------------------
def preprocess(edge_index: np.ndarray):
    """Static per-core edge tiling.  Returns per-core idx/dstl slot arrays
    ([128, T_total], slot s -> partition s%128, col s//128), the common
    per-window tile counts, and the padded per-node scale tables."""
    src = np.ascontiguousarray(edge_index[0]).astype(np.int64)
    dst = np.ascontiguousarray(edge_index[1]).astype(np.int64)

    deg_out = np.bincount(src, minlength=N_NODES).astype(np.float64)
    deg_in = np.bincount(dst, minlength=N_NODES).astype(np.float64)
    dinv_out = np.where(
        deg_out > 0, 1.0 / np.sqrt(np.maximum(deg_out, 1.0)), 0.0
    ).astype(np.float32)
    dinv_in = np.where(
        deg_in > 0, 1.0 / np.sqrt(np.maximum(deg_in, 1.0)), 0.0
    ).astype(np.float32)

    dinv_out_p = np.zeros(NP_PAD, np.float32)
    dinv_out_p[:N_NODES] = dinv_out
    dinv_in_p = np.zeros(NP_PAD, np.float32)
    dinv_in_p[:N_NODES] = dinv_in

    core_of = dst // PER
    per_core = []
    for c in range(N_CORES):
        sel = np.nonzero(core_of == c)[0]
        dl = (dst[sel] - c * PER).astype(np.int64)
        order = np.argsort(dl, kind="stable")
        per_core.append((src[sel][order], dl[order]))

    # per-(core, window) edge counts -> common per-window tile counts
    counts = np.zeros((N_CORES, N_WIN), np.int64)
    for c in range(N_CORES):
        _, dl = per_core[c]
        counts[c] = np.bincount(dl // WIN, minlength=N_WIN)
    tiles_per_win = np.maximum(1, (counts.max(axis=0) + 127) // 128)  # [N_WIN]
    t_starts = np.concatenate([[0], np.cumsum(tiles_per_win)])
    T_total = int(t_starts[-1])

    idx_all = np.zeros((N_CORES, 128, T_total), np.int32)
    dstl_all = np.full((N_CORES, 128, T_total), -1.0, np.float32)
    for c in range(N_CORES):
        s_c, dl_c = per_core[c]
        idx_flat = np.zeros(T_total * 128, np.int32)
        dstl_flat = np.full(T_total * 128, -1.0, np.float32)
        w_starts = np.concatenate([[0], np.cumsum(counts[c])])
        for w in range(N_WIN):
            lo, hi = w_starts[w], w_starts[w + 1]
            n = hi - lo
            base = int(t_starts[w]) * 128
            idx_flat[base : base + n] = s_c[lo:hi]
            dstl_flat[base : base + n] = (dl_c[lo:hi] - w * WIN).astype(
                np.float32
            )
        idx_all[c] = idx_flat.reshape(T_total, 128).T
        dstl_all[c] = dstl_flat.reshape(T_total, 128).T

    # per-core padded scale tables [128, BLOCKS]: node c*PER + 128*b + p
    def shard_table(vec):
        out = np.zeros((N_CORES, 128, BLOCKS), np.float32)
        for c in range(N_CORES):
            sh = vec[c * PER : (c + 1) * PER].reshape(BLOCKS, 128).T
            out[c] = sh
        return out

    do_sh = shard_table(dinv_out_p)
    di_sh = shard_table(dinv_in_p)
    cdd_sh = shard_table(dinv_in_p * dinv_out_p)

    return {
        "idx_all": idx_all,
        "dstl_all": dstl_all,
        "tiles_per_win": tiles_per_win,
        "t_starts": t_starts,
        "T_total": T_total,
        "do_sh": do_sh,
        "di_sh": di_sh,
        "cdd_sh": cdd_sh,
    }


# ----------------------------------------------------------------------------
def build_kernel(tiles_per_win, t_starts, T_total, n_hops=None):
    if n_hops is None:
        n_hops = K_HOPS
    nc = bass.Bass()

    x_sh = nc.dram_tensor("x_sh", [PER, IN_F], _f32, kind="ExternalInput")
    w1 = nc.dram_tensor("w1", [IN_F, HID], _f32, kind="ExternalInput")
    w2 = nc.dram_tensor("w2", [HID, OUT_F], _f32, kind="ExternalInput")
    s_rep = nc.dram_tensor("s_rep", [128, F], _f32, kind="ExternalInput")
    do_t = nc.dram_tensor("do_sh", [128, BLOCKS], _f32, kind="ExternalInput")
    di_t = nc.dram_tensor("di_sh", [128, BLOCKS], _f32, kind="ExternalInput")
    cdd_t = nc.dram_tensor("cdd_sh", [128, BLOCKS], _f32, kind="ExternalInput")
    idx_t = nc.dram_tensor("idx_t", [128, T_total], _i32, kind="ExternalInput")
    dstl_t = nc.dram_tensor("dstl_t", [128, T_total], _f32, kind="ExternalInput")
    out_sh = nc.dram_tensor("out_sh", [PER, F], _f32, kind="ExternalOutput")

    gsh = nc.dram_tensor("gsh", [PER, F], _f32)
    gfa = nc.dram_tensor("gfa", [NP_PAD, F], _f32)
    gfb = nc.dram_tensor("gfb", [NP_PAD, F], _f32)
    hst = nc.dram_tensor("hst", [BLOCKS, n_hops + 1, 128, F], _f32)

    groups = [list(range(N_CORES))]

    with tile.TileContext(nc) as tc:
        with (
            tc.tile_pool(name="const", bufs=1) as constp,
            tc.tile_pool(name="mlp", bufs=3) as mlpp,
            tc.tile_pool(name="mlppsum", bufs=2, space="PSUM") as mlpps,
            tc.tile_pool(name="hoppsum", bufs=2, space="PSUM") as hopps,
            tc.tile_pool(name="msg", bufs=12) as msgp,
            tc.tile_pool(name="oh", bufs=3) as ohp,
            tc.tile_pool(name="small", bufs=8) as smallp,
        ):
            # ---- constants in SBUF
            ident = constp.tile([128, 128], _f32)
            make_identity(nc, ident[:])
            iota = constp.tile([128, WIN], _f32)
            nc.gpsimd.iota(
                iota[:], pattern=[[1, WIN]], base=0, channel_multiplier=0,
                allow_small_or_imprecise_dtypes=True,
            )
            w1sb = constp.tile([128, 4 * HID], _f32)  # 4 k-tiles side by side
            for k in range(4):
                nc.sync.dma_start(
                    w1sb[:, k * HID : (k + 1) * HID],
                    w1[k * 128 : (k + 1) * 128, :],
                )
            w2sb = constp.tile([HID, OUT_F], _f32)
            nc.sync.dma_start(w2sb[:], w2[:])
            srepsb = constp.tile([128, F], _f32)
            nc.sync.dma_start(srepsb[:], s_rep[:])
            dosb = constp.tile([128, BLOCKS], _f32)
            nc.sync.dma_start(dosb[:], do_t[:])
            disb = constp.tile([128, BLOCKS], _f32)
            nc.sync.dma_start(disb[:], di_t[:])
            cddsb = constp.tile([128, BLOCKS], _f32)
            nc.sync.dma_start(cddsb[:], cdd_t[:])
            idxsb = constp.tile([128, T_total], _i32)
            nc.sync.dma_start(idxsb[:], idx_t[:])
            dstlsb = constp.tile([128, T_total], _f32)
            nc.sync.dma_start(dstlsb[:], dstl_t[:])

            # ---- Phase A: MLP over this core's shard
            for b in range(BLOCKS):
                xb = mlpp.tile([128, IN_F], _f32, tag="xb")
                nc.sync.dma_start(xb[:], x_sh[b * 128 : (b + 1) * 128, :])
                xT = mlpp.tile([128, IN_F], _f32, tag="xT")
                for k in range(4):
                    pst = mlpps.tile([128, 128], _f32, tag="pst")
                    nc.tensor.transpose(
                        pst[:], xb[:, k * 128 : (k + 1) * 128], ident[:]
                    )
                    nc.vector.tensor_copy(xT[:, k * 128 : (k + 1) * 128], pst[:])
                ph1 = mlpps.tile([HID, 128], _f32, tag="ph1")
                for k in range(4):
                    nc.tensor.matmul(
                        ph1[:],
                        lhsT=w1sb[:, k * HID : (k + 1) * HID],
                        rhs=xT[:, k * 128 : (k + 1) * 128],
                        start=(k == 0),
                        stop=(k == 3),
                    )
                h1T = mlpp.tile([HID, 128], _f32, tag="h1T")
                nc.scalar.activation(
                    h1T[:], ph1[:], mybir.ActivationFunctionType.Relu
                )
                ph2 = mlpps.tile([128, OUT_F], _f32, tag="ph2")
                nc.tensor.matmul(
                    ph2[:], lhsT=h1T[:], rhs=w2sb[:], start=True, stop=True
                )
                hb = mlpp.tile([128, F], _f32, tag="hb")
                nc.scalar.activation(
                    hb[:], ph2[:], mybir.ActivationFunctionType.Relu
                )
                nc.sync.dma_start(hst[b, 0], hb[:])
                gb = mlpp.tile([128, F], _f32, tag="gb")
                nc.vector.tensor_scalar_mul(gb[:], hb[:], dosb[:, b : b + 1])
                nc.sync.dma_start(gsh[b * 128 : (b + 1) * 128, :], gb[:])

            nc.gpsimd.collective_compute(
                "AllGather", mybir.AluOpType.bypass, replica_groups=groups,
                ins=[gsh[:]], outs=[gfa[:]],
            )

            # ---- Phase B: propagation hops
            for k in range(1, n_hops + 1):
                gin = gfa if (k % 2 == 1) else gfb
                gout = gfb if (k % 2 == 1) else gfa
                for w in range(N_WIN):
                    t0, tw = int(t_starts[w]), int(tiles_per_win[w])
                    # batched one-hot for the whole window: [128, tw, WIN]
                    ohw = ohp.tile([128, tw * WIN], _f32, tag="ohw")
                    a = dstlsb[:, t0 : t0 + tw]
                    in0 = AP(a.tensor, a.offset, [a.ap[0], a.ap[1], [0, WIN]])
                    i_ = iota[:]
                    in1 = AP(i_.tensor, i_.offset, [i_.ap[0], [0, tw], i_.ap[1]])
                    o_ = ohw[:]
                    outap = AP(o_.tensor, o_.offset, [o_.ap[0], [WIN, tw], [1, WIN]])
                    nc.vector.tensor_tensor(
                        out=outap, in0=in0, in1=in1, op=mybir.AluOpType.is_equal
                    )
                    ps = hopps.tile([WIN, F], _f32, tag="ps")
                    for i in range(tw):
                        t = t0 + i
                        msg = msgp.tile([128, F], _f32, tag="msg")
                        nc.gpsimd.indirect_dma_start(
                            out=msg[:],
                            out_offset=None,
                            in_=gin[:, :],
                            in_offset=IndirectOffsetOnAxis(
                                ap=idxsb[:, t : t + 1], axis=0
                            ),
                        )
                        nc.tensor.matmul(
                            ps[:],
                            lhsT=ohw[:, i * WIN : (i + 1) * WIN],
                            rhs=msg[:],
                            start=(i == 0),
                            stop=(i == tw - 1),
                        )
                    b, half = w // 2, w % 2
                    hk = smallp.tile([WIN, F], _f32, tag="hk")
                    nc.vector.tensor_scalar_mul(
                        hk[:], ps[:], disb[half * WIN : (half + 1) * WIN, b : b + 1]
                    )
                    nc.sync.dma_start(
                        hst[b, k, half * WIN : (half + 1) * WIN, :], hk[:]
                    )
                    gk = smallp.tile([WIN, F], _f32, tag="gk")
                    nc.vector.tensor_scalar_mul(
                        gk[:], ps[:], cddsb[half * WIN : (half + 1) * WIN, b : b + 1]
                    )
                    nc.sync.dma_start(
                        gsh[w * WIN : (w + 1) * WIN, :], gk[:]
                    )
                if k < n_hops:
                    nc.gpsimd.collective_compute(
                        "AllGather", mybir.AluOpType.bypass,
                        replica_groups=groups, ins=[gsh[:]], outs=[gout[:]],
                    )

            # ---- Phase C: adaptive gate
            KH = n_hops + 1
            for b in range(BLOCKS):
                hbt = mlpp.tile([128, KH * F], _f32, tag="hbt")
                hsrc = hst[b]  # [KH, 128, F]
                hap = AP(hsrc.tensor, hsrc.offset, [[F, 128], [128 * F, KH], [1, F]])
                nc.sync.dma_start(hbt[:], hap)
                tmp = mlpp.tile([128, KH * F], _f32, tag="tmp")
                hb3 = AP(hbt[:].tensor, hbt[:].offset, [hbt[:].ap[0], [F, KH], [1, F]])
                sr = srepsb[:]
                sr3 = AP(sr.tensor, sr.offset, [sr.ap[0], [0, KH], [1, F]])
                t3 = AP(tmp[:].tensor, tmp[:].offset, [tmp[:].ap[0], [F, KH], [1, F]])
                nc.vector.tensor_tensor(
                    out=t3, in0=hb3, in1=sr3, op=mybir.AluOpType.mult
                )
                sc = smallp.tile([128, KH], _f32, tag="sc")
                nc.vector.tensor_reduce(
                    sc[:], t3, axis=mybir.AxisListType.X, op=mybir.AluOpType.add
                )
                scs = smallp.tile([128, KH], _f32, tag="scs")
                nc.scalar.activation(
                    scs[:], sc[:], mybir.ActivationFunctionType.Sigmoid
                )
                tmp2 = mlpp.tile([128, KH * F], _f32, tag="tmp2")
                s3 = AP(scs[:].tensor, scs[:].offset, [scs[:].ap[0], [1, KH], [0, F]])
                t23 = AP(tmp2[:].tensor, tmp2[:].offset, [tmp2[:].ap[0], [F, KH], [1, F]])
                nc.vector.tensor_tensor(
                    out=t23, in0=hb3, in1=s3, op=mybir.AluOpType.mult
                )
                ob = smallp.tile([128, F], _f32, tag="ob")
                t2r = AP(tmp2[:].tensor, tmp2[:].offset, [tmp2[:].ap[0], [1, F], [F, KH]])
                nc.vector.tensor_reduce(
                    ob[:], t2r, axis=mybir.AxisListType.X, op=mybir.AluOpType.add
                )
                nc.sync.dma_start(out_sh[b * 128 : (b + 1) * 128, :], ob[:])

    split_excess_waits(nc)
    return nc


# ----------------------------------------------------------------------------
_COMPILED = {}


def kernel(x, edge_index, W1, W2, s) -> np.ndarray:
    x = np.asarray(x, np.float32)
    edge_index = np.asarray(edge_index)
    W1 = np.asarray(W1, np.float32)
    W2 = np.asarray(W2, np.float32)
    s = np.asarray(s, np.float32)

    pp = preprocess(edge_index)

    nc = build_kernel(pp["tiles_per_win"], pp["t_starts"], pp["T_total"])

    x_pad = np.zeros((NP_PAD, IN_F), np.float32)
    x_pad[:N_NODES] = x
    s_rep = np.repeat(s.reshape(1, F), 128, axis=0).astype(np.float32)

    in_maps = []
    for c in range(N_CORES):
        in_maps.append(
            {
                "x_sh": np.ascontiguousarray(x_pad[c * PER : (c + 1) * PER]),
                "w1": W1,
                "w2": W2,
                "s_rep": s_rep,
                "do_sh": np.ascontiguousarray(pp["do_sh"][c]),
                "di_sh": np.ascontiguousarray(pp["di_sh"][c]),
                "cdd_sh": np.ascontiguousarray(pp["cdd_sh"][c]),
                "idx_t": np.ascontiguousarray(pp["idx_all"][c]),
                "dstl_t": np.ascontiguousarray(pp["dstl_all"][c]),
            }
        )

    res = run_bass_kernel_spmd(nc, in_maps, list(range(N_CORES)))
    out = np.concatenate(
        [res.results[c]["out_sh"] for c in range(N_CORES)], axis=0
    )
    return np.ascontiguousarray(out[:N_NODES])


if __name__ == "__main__":
    rng = np.random.default_rng(0)
    x = rng.standard_normal((N_NODES, IN_F), dtype=np.float32)
    ei = rng.integers(0, N_NODES, (2, N_EDGES)).astype(np.int64)
    W1 = rng.standard_normal((IN_F, HID), dtype=np.float32) / np.sqrt(IN_F)
    W2 = rng.standard_normal((HID, OUT_F), dtype=np.float32) / np.sqrt(HID)
    s = rng.standard_normal((OUT_F, 1), dtype=np.float32) / np.sqrt(OUT_F)
    out = kernel(x=x, edge_index=ei, W1=W1, W2=W2, s=s)
    print("out", out.shape, out.dtype, float(np.abs(out).mean()))
